# revision 7
# baseline (speedup 1.0000x reference)
"""PairRepresentation kernel for 8x Trainium2 NeuronCores (Bass/Tile).

Math: out[b,i,j,:] = (left[i] + right[j] + E[j-i+2048]) @ Wo + bo
with left = x@Wl + bl, right = x@Wr + br.

Since the projection distributes over the sum:
  out[i,j] = (x@Wl)[i]@Wo + (x@Wr)[j]@Wo + E[j-i+2048]@Wo + ((bl+br)@Wo + bo)

Each core owns 96 consecutive i-rows (sequence parallel over the first L
axis). On device, everything is kept channel-on-partitions:
  RpT [256, 768]  = Wo^T @ (x@Wr)^T          (all j, replicated work)
  LpT [256, 96]   = Wo^T @ (x@Wl)^T[:, own] + all biases
  EpT [256, 864]  = Wo^T @ E_window^T        (per-core 863-wide rel window)
Then one fused DVE scalar_tensor_tensor per (i, ch-half):
  S[c, j] = (EpT[c, 95-i+j] + LpT[c, i]) + RpT[c, j]
and batched 3MB DMA writes to a transposed DRAM output outT[c, i, j].
The host gather transposes back to (1, L, L, 256).
"""

import sys

for p in ("/opt/trn_rl_repo", "/root/.axon_site/_ro/trn_rl_repo"):
    if p not in sys.path:
        sys.path.append(p)

import numpy as np

import concourse.bass as bass
import concourse.tile as tile
from concourse import bacc, mybir
from concourse.bass import ts
from concourse.bass_utils import run_bass_kernel_spmd

N_CORES = 8
L = 768
D = 256
DP = 64
MAX_LEN = 2048
LPC = L // N_CORES  # 96 i-rows per core
IB = 8              # i-rows per output batch/DMA
NB = LPC // IB      # 12 batches
EW = LPC - 1 + L    # 863: per-core rel-pos window width
EWP = 864           # padded to even
F32 = mybir.dt.float32

_CACHED_NC = None
_last_in_maps = None


def _build_nc():
    nc = bacc.Bacc("TRN2", target_bir_lowering=False, debug=False,
                   num_devices=N_CORES)

    xT_d = nc.dram_tensor("xT", [D, L], F32, kind="ExternalInput")
    xo_d = nc.dram_tensor("xT_own", [D, LPC], F32, kind="ExternalInput")
    Wl_d = nc.dram_tensor("Wl", [D, DP], F32, kind="ExternalInput")
    Wr_d = nc.dram_tensor("Wr", [D, DP], F32, kind="ExternalInput")
    Wo_d = nc.dram_tensor("Wo", [DP, D], F32, kind="ExternalInput")
    blbr_d = nc.dram_tensor("blbr", [DP, 1], F32, kind="ExternalInput")
    bo2_d = nc.dram_tensor("bo2", [D, 1], F32, kind="ExternalInput")
    ew_d = nc.dram_tensor("EwT", [DP, EWP], F32, kind="ExternalInput")
    out_d = nc.dram_tensor("outT", [D, LPC, L], F32, kind="ExternalOutput")

    ADD = mybir.AluOpType.add

    with tile.TileContext(nc) as tc:
        with (
            tc.tile_pool(name="consts", bufs=1) as cp,
            tc.tile_pool(name="psum", bufs=2, space=bass.MemorySpace.PSUM) as pp,
            tc.tile_pool(name="work", bufs=2) as wp,
        ):
            # ---- load inputs (channel halves on partitions) ----
            # order matters: the rr -> Rp/Ep chain gates the main loop.
            xTt = [cp.tile([128, L], F32, name=f"xT{h}", tag=f"xT{h}") for h in range(2)]
            xot = [cp.tile([128, LPC], F32, name=f"xo{h}", tag=f"xo{h}") for h in range(2)]
            Wlt = [cp.tile([128, DP], F32, name=f"Wl{h}", tag=f"Wl{h}") for h in range(2)]
            Wrt = [cp.tile([128, DP], F32, name=f"Wr{h}", tag=f"Wr{h}") for h in range(2)]
            bo2t = [cp.tile([128, 1], F32, name=f"bo{h}", tag=f"bo{h}") for h in range(2)]
            Wot = cp.tile([DP, D], F32)
            blbrt = cp.tile([DP, 1], F32)
            ewt = cp.tile([DP, EWP], F32)
            for h in range(2):
                nc.sync.dma_start(out=Wrt[h][:], in_=Wr_d[ts(h, 128), :])
                nc.sync.dma_start(out=xTt[h][:], in_=xT_d[ts(h, 128), :])
            nc.sync.dma_start(out=Wot[:], in_=Wo_d[:])
            nc.sync.dma_start(out=ewt[:], in_=ew_d[:])
            for h in range(2):
                nc.sync.dma_start(out=Wlt[h][:], in_=Wl_d[ts(h, 128), :])
                nc.sync.dma_start(out=xot[h][:], in_=xo_d[ts(h, 128), :])
                nc.sync.dma_start(out=bo2t[h][:], in_=bo2_d[ts(h, 128), :])
            nc.sync.dma_start(out=blbrt[:], in_=blbr_d[:])

            # ---- stage 1+2: tiny projection chain on PE ----
            # rrT = (x@Wr)^T [64, 768], then RpT/EpT (gate the main loop),
            # then the left chain (only gates the ACT bias pass).
            rrT = cp.tile([DP, L], F32)
            for n in range(0, L, 384):
                ps = pp.tile([DP, 384], F32, tag="ps")
                nc.tensor.matmul(ps[:], Wrt[0][:], xTt[0][:, n:n + 384],
                                 start=True, stop=False)
                nc.tensor.matmul(ps[:], Wrt[1][:], xTt[1][:, n:n + 384],
                                 start=False, stop=True)
                nc.scalar.copy(rrT[:, n:n + 384], ps[:])

            RpT = [cp.tile([128, L], F32, name=f"Rp{h}", tag=f"Rp{h}") for h in range(2)]
            LpT = [cp.tile([128, LPC], F32, name=f"Lp{h}", tag=f"Lp{h}") for h in range(2)]
            EpT = [cp.tile([128, EWP], F32, name=f"Ep{h}", tag=f"Ep{h}") for h in range(2)]
            for h in range(2):
                woh = Wot[:, ts(h, 128)]  # lhsT [64, 128]
                for n in range(0, L, 384):
                    ps = pp.tile([128, 384], F32, tag="ps")
                    nc.tensor.matmul(ps[:], woh, rrT[:, n:n + 384],
                                     start=True, stop=True)
                    nc.scalar.copy(RpT[h][:, n:n + 384], ps[:])
                for n in range(0, EWP, 432):
                    ps = pp.tile([128, 432], F32, tag="ps")
                    nc.tensor.matmul(ps[:], woh, ewt[:, n:n + 432],
                                     start=True, stop=True)
                    nc.scalar.copy(EpT[h][:, n:n + 432], ps[:])

            lrT = cp.tile([DP, LPC], F32)
            ps = pp.tile([DP, LPC], F32, tag="ps")
            nc.tensor.matmul(ps[:], Wlt[0][:], xot[0][:], start=True, stop=False)
            nc.tensor.matmul(ps[:], Wlt[1][:], xot[1][:], start=False, stop=True)
            # fold bl+br here: (leftRaw + bl + br) @ Wo
            nc.scalar.add(lrT[:], ps[:], add=blbrt[:, 0:1])
            for h in range(2):
                ps = pp.tile([128, LPC], F32, tag="ps")
                nc.tensor.matmul(ps[:], Wot[:, ts(h, 128)], lrT[:],
                                 start=True, stop=True)
                # fold bo here
                nc.scalar.add(LpT[h][:], ps[:], add=bo2t[h][:, 0:1])

            # ---- stage 3: batched pair sum + output DMA ----
            # One DVE tensor_tensor per (batch, ch-half) covering IB i-rows:
            #   in0 = EpT diagonal view  [128, (i: step -1), (j: step 1)]
            #   in1 = RpT broadcast view [128, (i: step 0),  (j: step 1)]
            # then per-i in-place bias add of LpT[:, i] on the Scalar engine.
            for b in range(NB):
                for h in range(2):
                    S = wp.tile([128, IB, L], F32, name=f"S{h}", tag=f"S{h}")
                    base = EpT[h][:, LPC - 1 - b * IB:]
                    ep_diag = bass.AP(
                        base.tensor, base.offset,
                        [list(base.ap[0]), [-1, IB], [1, L]])
                    rp_bcast = RpT[h][:, None, :].broadcast_to([128, IB, L])
                    nc.vector.tensor_add(S[:], ep_diag, rp_bcast)
                    qn = IB // 2
                    for q in range(2):
                        for bi in range(q * qn, (q + 1) * qn):
                            il = b * IB + bi
                            nc.scalar.add(S[:, bi, :], S[:, bi, :],
                                          add=LpT[h][:, il:il + 1])
                        # issue each half's DMA as soon as its rows are
                        # biased; alternate the two HWDGE rings (SP/ACT)
                        dma_eng = nc.sync if (b + h + q) % 2 == 0 else nc.scalar
                        dma_eng.dma_start(
                            out=out_d[ts(h, 128), b * IB + q * qn:
                                      b * IB + (q + 1) * qn, :],
                            in_=S[:, q * qn:(q + 1) * qn, :])

    nc.compile()
    return nc


def kernel(x, Wl, bl, Wr, br, E, Wo, bo):
    global _CACHED_NC
    x = np.asarray(x, dtype=np.float32)
    Wl = np.asarray(Wl, dtype=np.float32)
    bl = np.asarray(bl, dtype=np.float32)
    Wr = np.asarray(Wr, dtype=np.float32)
    br = np.asarray(br, dtype=np.float32)
    E = np.asarray(E, dtype=np.float32)
    Wo = np.asarray(Wo, dtype=np.float32)
    bo = np.asarray(bo, dtype=np.float32)

    B = x.shape[0]
    assert x.shape == (B, L, D) and B == 1

    xT = np.ascontiguousarray(x[0].T)                       # (256, 768)
    # rel index range used: E rows [2048-767, 2048+767] = [1281, 2815]
    EwT = np.ascontiguousarray(E[MAX_LEN - (L - 1):MAX_LEN + L].T)  # (64, 1535)
    blbr = np.ascontiguousarray((bl + br).reshape(DP, 1))
    bo2 = np.ascontiguousarray(bo.reshape(D, 1))

    in_maps = []
    for c in range(N_CORES):
        i0 = c * LPC
        # core c needs Ew columns w = j - i + (L-1) for i in [i0, i0+96),
        # j in [0, 768)  ->  w in [s0, s0 + 863), s0 = (L-1) - i0 - (LPC-1)
        s0 = (L - 1) - i0 - (LPC - 1)
        ewc = np.zeros((DP, EWP), dtype=np.float32)
        ewc[:, :EW] = EwT[:, s0:s0 + EW]
        in_maps.append({
            "xT": xT,
            "xT_own": np.ascontiguousarray(xT[:, i0:i0 + LPC]),
            "Wl": Wl, "Wr": Wr, "Wo": Wo,
            "blbr": blbr, "bo2": bo2,
            "EwT": ewc,
        })

    global _last_in_maps
    _last_in_maps = in_maps

    if _CACHED_NC is None:
        _CACHED_NC = _build_nc()
    nc = _CACHED_NC

    res = run_bass_kernel_spmd(nc, in_maps, list(range(N_CORES)))
    # per-core outT: (256, 96, 768) = [c, i_local, j]
    full = np.concatenate([res.results[c]["outT"] for c in range(N_CORES)],
                          axis=1)                            # (256, 768, 768)
    return np.ascontiguousarray(full.transpose(1, 2, 0))[None]  # (1,768,768,256)


# revision 8
# speedup vs baseline: 1.1045x; 1.1045x over previous
"""PairRepresentation kernel for 8x Trainium2 NeuronCores (Bass/Tile).

Math: out[b,i,j,:] = (left[i] + right[j] + E[j-i+2048]) @ Wo + bo
with left = x@Wl + bl, right = x@Wr + br.

Since the projection distributes over the sum:
  out[i,j] = (x@Wl)[i]@Wo + (x@Wr)[j]@Wo + E[j-i+2048]@Wo + ((bl+br)@Wo + bo)

Each core owns 96 consecutive i-rows (sequence parallel over the first L
axis). On device, everything is kept channel-on-partitions:
  RpT [256, 768]  = Wo^T @ (x@Wr)^T          (all j, replicated work)
  LpT [256, 96]   = Wo^T @ (x@Wl)^T[:, own] + all biases
  EpT [256, 864]  = Wo^T @ E_window^T        (per-core 863-wide rel window)
Then one fused DVE scalar_tensor_tensor per (i, ch-half):
  S[c, j] = (EpT[c, 95-i+j] + LpT[c, i]) + RpT[c, j]
and batched 3MB DMA writes to a transposed DRAM output outT[c, i, j].
The host gather transposes back to (1, L, L, 256).
"""

import sys

for p in ("/opt/trn_rl_repo", "/root/.axon_site/_ro/trn_rl_repo"):
    if p not in sys.path:
        sys.path.append(p)

import numpy as np

import concourse.bass as bass
import concourse.tile as tile
from concourse import bacc, mybir
from concourse.bass import ts
from concourse.bass_utils import run_bass_kernel_spmd

N_CORES = 8
L = 768
D = 256
DP = 64
MAX_LEN = 2048
LPC = L // N_CORES  # 96 i-rows per core
IB = 8              # i-rows per output batch/DMA
NB = LPC // IB      # 12 batches
EW = LPC - 1 + L    # 863: per-core rel-pos window width
EWP = 864           # padded to even
F32 = mybir.dt.float32

_CACHED_NC = None
_last_in_maps = None


def _build_nc():
    nc = bacc.Bacc("TRN2", target_bir_lowering=False, debug=False,
                   num_devices=N_CORES)

    xT_d = nc.dram_tensor("xT", [D, L], F32, kind="ExternalInput")
    xo_d = nc.dram_tensor("xT_own", [D, LPC], F32, kind="ExternalInput")
    Wl_d = nc.dram_tensor("Wl", [D, DP], F32, kind="ExternalInput")
    Wr_d = nc.dram_tensor("Wr", [D, DP], F32, kind="ExternalInput")
    Wo_d = nc.dram_tensor("Wo", [DP, D], F32, kind="ExternalInput")
    blbr_d = nc.dram_tensor("blbr", [DP, 1], F32, kind="ExternalInput")
    bo2_d = nc.dram_tensor("bo2", [D, 1], F32, kind="ExternalInput")
    ew_d = nc.dram_tensor("EwT", [DP, EWP], F32, kind="ExternalInput")
    out_d = nc.dram_tensor("outT", [D, LPC, L], F32, kind="ExternalOutput")

    ADD = mybir.AluOpType.add

    with tile.TileContext(nc) as tc:
        with (
            tc.tile_pool(name="consts", bufs=1) as cp,
            tc.tile_pool(name="psum", bufs=2, space=bass.MemorySpace.PSUM) as pp,
            tc.tile_pool(name="work", bufs=2) as wp,
        ):
            # ---- load inputs (channel halves on partitions) ----
            # order matters: the rr -> Rp/Ep chain gates the main loop.
            xTt = [cp.tile([128, L], F32, name=f"xT{h}", tag=f"xT{h}") for h in range(2)]
            xot = [cp.tile([128, LPC], F32, name=f"xo{h}", tag=f"xo{h}") for h in range(2)]
            Wlt = [cp.tile([128, DP], F32, name=f"Wl{h}", tag=f"Wl{h}") for h in range(2)]
            Wrt = [cp.tile([128, DP], F32, name=f"Wr{h}", tag=f"Wr{h}") for h in range(2)]
            bo2t = [cp.tile([128, 1], F32, name=f"bo{h}", tag=f"bo{h}") for h in range(2)]
            Wot = cp.tile([DP, D], F32)
            blbrt = cp.tile([DP, 1], F32)
            ewt = cp.tile([DP, EWP], F32)
            for h in range(2):
                nc.sync.dma_start(out=Wrt[h][:], in_=Wr_d[ts(h, 128), :])
                nc.sync.dma_start(out=xTt[h][:], in_=xT_d[ts(h, 128), :])
            nc.sync.dma_start(out=Wot[:], in_=Wo_d[:])
            nc.sync.dma_start(out=ewt[:], in_=ew_d[:])
            for h in range(2):
                nc.sync.dma_start(out=Wlt[h][:], in_=Wl_d[ts(h, 128), :])
                nc.sync.dma_start(out=xot[h][:], in_=xo_d[ts(h, 128), :])
                nc.sync.dma_start(out=bo2t[h][:], in_=bo2_d[ts(h, 128), :])
            nc.sync.dma_start(out=blbrt[:], in_=blbr_d[:])

            # ---- stage 1+2: tiny projection chain on PE ----
            # rrT = (x@Wr)^T [64, 768], then RpT/EpT (gate the main loop),
            # then the left chain (only gates the ACT bias pass).
            rrT = cp.tile([DP, L], F32)
            for n in range(0, L, 384):
                ps = pp.tile([DP, 384], F32, tag="ps")
                nc.tensor.matmul(ps[:], Wrt[0][:], xTt[0][:, n:n + 384],
                                 start=True, stop=False)
                nc.tensor.matmul(ps[:], Wrt[1][:], xTt[1][:, n:n + 384],
                                 start=False, stop=True)
                nc.scalar.copy(rrT[:, n:n + 384], ps[:])

            RpT = [cp.tile([128, L], F32, name=f"Rp{h}", tag=f"Rp{h}") for h in range(2)]
            LpT = [cp.tile([128, LPC], F32, name=f"Lp{h}", tag=f"Lp{h}") for h in range(2)]
            EpT = [cp.tile([128, EWP], F32, name=f"Ep{h}", tag=f"Ep{h}") for h in range(2)]
            for h in range(2):
                woh = Wot[:, ts(h, 128)]  # lhsT [64, 128]
                for n in range(0, L, 384):
                    ps = pp.tile([128, 384], F32, tag="ps")
                    nc.tensor.matmul(ps[:], woh, rrT[:, n:n + 384],
                                     start=True, stop=True)
                    nc.scalar.copy(RpT[h][:, n:n + 384], ps[:])
                for n in range(0, EWP, 432):
                    ps = pp.tile([128, 432], F32, tag="ps")
                    nc.tensor.matmul(ps[:], woh, ewt[:, n:n + 432],
                                     start=True, stop=True)
                    nc.scalar.copy(EpT[h][:, n:n + 432], ps[:])

            lrT = cp.tile([DP, LPC], F32)
            ps = pp.tile([DP, LPC], F32, tag="ps")
            nc.tensor.matmul(ps[:], Wlt[0][:], xot[0][:], start=True, stop=False)
            nc.tensor.matmul(ps[:], Wlt[1][:], xot[1][:], start=False, stop=True)
            # fold bl+br here: (leftRaw + bl + br) @ Wo
            nc.scalar.add(lrT[:], ps[:], add=blbrt[:, 0:1])
            for h in range(2):
                ps = pp.tile([128, LPC], F32, tag="ps")
                nc.tensor.matmul(ps[:], Wot[:, ts(h, 128)], lrT[:],
                                 start=True, stop=True)
                # fold bo here
                nc.scalar.add(LpT[h][:], ps[:], add=bo2t[h][:, 0:1])

            # ---- stage 3: batched pair sum + output DMA ----
            # One DVE tensor_tensor per (batch, ch-half) covering IB i-rows:
            #   in0 = EpT diagonal view  [128, (i: step -1), (j: step 1)]
            #   in1 = RpT broadcast view [128, (i: step 0),  (j: step 1)]
            # then per-i in-place bias add of LpT[:, i] on the Scalar engine.
            for b in range(NB):
                for h in range(2):
                    S = wp.tile([128, IB, L], F32, name=f"S{h}", tag=f"S{h}")
                    base = EpT[h][:, LPC - 1 - b * IB:]
                    ep_diag = bass.AP(
                        base.tensor, base.offset,
                        [list(base.ap[0]), [-1, IB], [1, L]])
                    rp_bcast = RpT[h][:, None, :].broadcast_to([128, IB, L])
                    nc.vector.tensor_add(S[:], ep_diag, rp_bcast)
                    qn = IB // 2
                    for q in range(2):
                        for bi in range(q * qn, (q + 1) * qn):
                            il = b * IB + bi
                            nc.scalar.add(S[:, bi, :], S[:, bi, :],
                                          add=LpT[h][:, il:il + 1])
                        # issue each half's DMA as soon as its rows are
                        # biased (single HWDGE ring: two rings fragment
                        # the HBM write stream and cost ~19% bandwidth)
                        nc.sync.dma_start(
                            out=out_d[ts(h, 128), b * IB + q * qn:
                                      b * IB + (q + 1) * qn, :],
                            in_=S[:, q * qn:(q + 1) * qn, :])

    nc.compile()
    return nc


def kernel(x, Wl, bl, Wr, br, E, Wo, bo):
    global _CACHED_NC
    x = np.asarray(x, dtype=np.float32)
    Wl = np.asarray(Wl, dtype=np.float32)
    bl = np.asarray(bl, dtype=np.float32)
    Wr = np.asarray(Wr, dtype=np.float32)
    br = np.asarray(br, dtype=np.float32)
    E = np.asarray(E, dtype=np.float32)
    Wo = np.asarray(Wo, dtype=np.float32)
    bo = np.asarray(bo, dtype=np.float32)

    B = x.shape[0]
    assert x.shape == (B, L, D) and B == 1

    xT = np.ascontiguousarray(x[0].T)                       # (256, 768)
    # rel index range used: E rows [2048-767, 2048+767] = [1281, 2815]
    EwT = np.ascontiguousarray(E[MAX_LEN - (L - 1):MAX_LEN + L].T)  # (64, 1535)
    blbr = np.ascontiguousarray((bl + br).reshape(DP, 1))
    bo2 = np.ascontiguousarray(bo.reshape(D, 1))

    in_maps = []
    for c in range(N_CORES):
        i0 = c * LPC
        # core c needs Ew columns w = j - i + (L-1) for i in [i0, i0+96),
        # j in [0, 768)  ->  w in [s0, s0 + 863), s0 = (L-1) - i0 - (LPC-1)
        s0 = (L - 1) - i0 - (LPC - 1)
        ewc = np.zeros((DP, EWP), dtype=np.float32)
        ewc[:, :EW] = EwT[:, s0:s0 + EW]
        in_maps.append({
            "xT": xT,
            "xT_own": np.ascontiguousarray(xT[:, i0:i0 + LPC]),
            "Wl": Wl, "Wr": Wr, "Wo": Wo,
            "blbr": blbr, "bo2": bo2,
            "EwT": ewc,
        })

    global _last_in_maps
    _last_in_maps = in_maps

    if _CACHED_NC is None:
        _CACHED_NC = _build_nc()
    nc = _CACHED_NC

    res = run_bass_kernel_spmd(nc, in_maps, list(range(N_CORES)))
    # per-core outT: (256, 96, 768) = [c, i_local, j]
    full = np.concatenate([res.results[c]["outT"] for c in range(N_CORES)],
                          axis=1)                            # (256, 768, 768)
    return np.ascontiguousarray(full.transpose(1, 2, 0))[None]  # (1,768,768,256)
